# revision 39
# baseline (speedup 1.0000x reference)
"""Multi-head latent attention (MLA) Trainium2 kernel, 8-core SPMD, v2.

Sharding: 2 batch-groups of 4 cores (cores 0-3 = batch 0, 4-7 = batch 1).
Within a group, core w owns heads {4w..4w+3} and, at the end, token rows
{128w+128j} of each span's ReduceScatter output.

Design (fully span-pipelined, no mid-kernel collective):
  - Replicate the latent down-projections per core over its whole batch:
    the fp8 ones are nearly free with DoubleRow, and the bf16 lv fills
    engine gaps in the attention phase; an AllGather variant measured
    slower (the transfer can't hide behind the cheap fp8 linear phase).
  - fp8e4m3 + DoubleRow matmuls (2 K-chunks per instruction) for the whole
    q/k/score path: lq down-proj, pos_k down-proj, fused k (x @ (Wkv_k@Wkup)),
    q up-proj, q_pos up-proj, and the score matmuls (main+positional parts
    paired into one DoubleRow instruction with zero-padded pos rows).
  - bf16 for the v path (lv down-proj, v up-proj), attention p@v, and o_proj:
    fp8 there fails the 2e-2 error budget (measured on the real inputs).
  - All biases folded host-side; b_vup routes through o_proj exactly as
    (b_vup @ w_o) added to b_o since softmax rows sum to 1.
  - Per-span partial o_proj + per-span ReduceScatter (4 small collectives
    that overlap compute; only the last one's ~28us is exposed).
RoPE via pre-permuted weight copies: rope(u) = u*cos + perm(u)*sin_signed.
Softmax is max-free (scores bounded): p = exp(s*scale), denominators via
ones-column matmul, normalization after the v-matmul.
"""
import numpy as np
import ml_dtypes

import concourse.bacc as bacc
import concourse.mybir as mybir
import concourse.tile as tile
from concourse.bass_utils import run_bass_kernel_spmd
from concourse.tile import add_dep_helper


def _dep(a, b, reason):
    add_dep_helper(getattr(a, 'ins', a), getattr(b, 'ins', b), sync=False,
                   reason=reason)

F32 = mybir.dt.float32
BF16 = mybir.dt.bfloat16
FP8 = mybir.dt.float8e4
AF = mybir.ActivationFunctionType
OP = mybir.AluOpType
PM = mybir.MatmulPerfMode
BF = ml_dtypes.bfloat16
F8 = ml_dtypes.float8_e4m3

MODEL = 2048
LATENT = 512
NH = 16
HD = 128
PHD = 64
THETA = 50000.0
B = 2
S = 2048
NC = 8
G = 4              # cores per batch-group
TS = 512           # tokens per span
NS = S // TS       # 4 spans
HC = NH // G       # 4 heads per core
NM = MODEL // 128  # 16 K-chunks of the model dim
LJ = LATENT // 128  # 4 K-chunks of the latent dim
SCALE = 1.0 / float(np.sqrt(HD + PHD))

# fp8 scale factors
SX = 16.0   # x
SW = 64.0   # weights
SL = 16.0   # lq latents
SQ = 16.0   # q-side activations
SK = 16.0   # k-side activations
EXP_SCALE = SCALE / (SQ * SK)

_ROT = np.r_[32:64, 0:32]

_CACHE = {}


def _build(nob):
    nc = bacc.Bacc("TRN2", target_bir_lowering=False, debug=False,
                   num_devices=NC)

    xb8 = nc.dram_tensor("xb8", [128, NS * NM * TS], FP8,
                         kind="ExternalInput")
    xlo8 = nc.dram_tensor("xlo8", [128, NS * NM * TS], FP8,
                          kind="ExternalInput")
    wlq8 = nc.dram_tensor("wlq8", [128, LJ * NM * 128], FP8,
                          kind="ExternalInput")
    wpk8 = nc.dram_tensor("wpk8", [128, NM * 128], FP8, kind="ExternalInput")
    wkf8 = nc.dram_tensor("wkf8", [128, HC * NM * 128], FP8,
                          kind="ExternalInput")
    wlvh8 = nc.dram_tensor("wlvh8", [128, LJ * NM * 128], FP8,
                           kind="ExternalInput")
    wlvl8 = nc.dram_tensor("wlvl8", [128, LJ * NM * 128], FP8,
                           kind="ExternalInput")
    wq8 = nc.dram_tensor("wq8", [128, LJ * HC * 128], FP8,
                         kind="ExternalInput")
    wqp8 = nc.dram_tensor("wqp8", [128, LJ * 512], FP8, kind="ExternalInput")
    wvp = nc.dram_tensor("wvp", [128, LJ * 512], BF16, kind="ExternalInput")
    wo = nc.dram_tensor("wo", [128, HC * 4 * TS], BF16, kind="ExternalInput")
    sc2 = nc.dram_tensor("sc2", [128, 2 * S], BF16, kind="ExternalInput")
    scp = nc.dram_tensor("scp", [128, S], BF16, kind="ExternalInput")
    bcon = nc.dram_tensor("bcon", [128, 32], F32, kind="ExternalInput")
    bob = nc.dram_tensor("bob", [128, MODEL], BF16, kind="ExternalInput")
    tri = nc.dram_tensor("tri", [128, 128], BF16, kind="ExternalInput")
    out_sh = nc.dram_tensor("out_sh", [NS * 128, MODEL], BF16,
                            kind="ExternalOutput")

    # bcon column map: 0:4 lq bias (per j), 4 posk raw, 5 posk rot,
    # 8:12 kT bias (per h), 12:16 qT bias (per h), 16:20 qpos (p,rr)
    BLQ, BPK, BKT, BQT, BQP = 0, 4, 8, 12, 16

    groups = [[0, 1, 2, 3], [4, 5, 6, 7]]

    with tile.TileContext(nc) as tc:
        with (
            tc.tile_pool(name="const", bufs=1) as cpool,
            tc.tile_pool(name="work", bufs=1) as wpool,
            tc.tile_pool(name="psum", bufs=1, space="PSUM") as pspool,
            tc.tile_pool(name="dram", bufs=1, space="DRAM") as dram,
        ):
            # ---- constants + span-0 inputs, DMA-ordered by first use ----
            bcon_sb = cpool.tile([128, 32], F32, tag="bcon")
            nc.sync.dma_start(out=bcon_sb[:], in_=bcon.ap())
            wlq_sb = cpool.tile([128, LJ * NM, 128], FP8, tag="wlq")
            nc.sync.dma_start(out=wlq_sb[:, 0:NM, :],
                              in_=wlq8.ap()[:, 0:NM * 128])
            x8t = [wpool.tile([128, NM, TS], FP8, tag="x8", bufs=2,
                              name=f"x8_{s}") for s in range(NS)]
            xlt = [wpool.tile([128, NM, TS], FP8, tag="xl", bufs=2,
                              name=f"xl_{s}") for s in range(NS)]

            def load_x8(s):
                for ch in range(4):
                    nc.sync.dma_start(
                        out=x8t[s][:, 4 * ch:4 * (ch + 1), :],
                        in_=xb8.ap()[:, NM * TS * s + 4 * TS * ch:
                                     NM * TS * s + 4 * TS * (ch + 1)])

            def load_xl(s):
                for ch in range(4):
                    nc.sync.dma_start(
                        out=xlt[s][:, 4 * ch:4 * (ch + 1), :],
                        in_=xlo8.ap()[:, NM * TS * s + 4 * TS * ch:
                                      NM * TS * s + 4 * TS * (ch + 1)])

            for ch in range(4):
                nc.sync.dma_start(
                    out=x8t[0][:, 4 * ch:4 * (ch + 1), :],
                    in_=xb8.ap()[:, 4 * TS * ch:4 * TS * (ch + 1)])
            nc.sync.dma_start(out=wlq_sb[:, NM:LJ * NM, :],
                              in_=wlq8.ap()[:, NM * 128:LJ * NM * 128])
            wkf_sb = cpool.tile([128, HC * NM, 128], FP8, tag="wkf")
            nc.sync.dma_start(out=wkf_sb[:], in_=wkf8.ap())
            wpk_sb = cpool.tile([128, NM, 128], FP8, tag="wpk")
            nc.sync.dma_start(out=wpk_sb[:], in_=wpk8.ap())
            scp_sb = cpool.tile([128, S], BF16, tag="scp")
            nc.sync.dma_start(out=scp_sb[:], in_=scp.ap())
            for ch in range(4):
                nc.sync.dma_start(
                    out=xlt[0][:, 4 * ch:4 * (ch + 1), :],
                    in_=xlo8.ap()[:, 4 * TS * ch:4 * TS * (ch + 1)])
            wlvh_sb = cpool.tile([128, LJ * NM, 128], FP8, tag="wlvh")
            nc.sync.dma_start(out=wlvh_sb[:], in_=wlvh8.ap())
            wlvl_sb = cpool.tile([128, LJ * NM, 128], FP8, tag="wlvl")
            nc.sync.dma_start(out=wlvl_sb[:], in_=wlvl8.ap())
            load_x8(1)
            wq_sb = cpool.tile([128, LJ, HC * 128], FP8, tag="wq")
            nc.sync.dma_start(out=wq_sb[:], in_=wq8.ap())
            wqp_sb = cpool.tile([128, LJ, 512], FP8, tag="wqp")
            nc.sync.dma_start(out=wqp_sb[:], in_=wqp8.ap())
            sc2_sb = cpool.tile([128, 2 * S], BF16, tag="sc2")
            nc.sync.dma_start(out=sc2_sb[:], in_=sc2.ap())
            wvp_sb = cpool.tile([128, LJ, 512], BF16, tag="wvp")
            nc.sync.dma_start(out=wvp_sb[:], in_=wvp.ap())
            tri_sb = cpool.tile([128, 128], BF16, tag="tri")
            wo_sb = cpool.tile([128, HC * 4, TS], BF16, tag="wo")
            bob_sb = cpool.tile([128, MODEL], BF16, tag="bob")
            ones_col = cpool.tile([128, 1], BF16, tag="onesc")
            nc.vector.memset(ones_col[:], 1.0)
            ones_row = cpool.tile([1, 128], BF16, tag="onesr")
            nc.vector.memset(ones_row[:], 1.0)

            # full-sequence k tensors: [feat 128, i(main/pos), S]
            kTz = [cpool.tile([128, 2, S], FP8, tag=f"kTz{h}",
                              name=f"kTz{h}") for h in range(HC)]
            for h in range(HC):
                # zero the unused pos rows once; q-side pos rows are garbage
                # and get multiplied by these zeros
                nc.vector.memset(kTz[h][64:128, 1, :], 0.0)
            v4 = [cpool.tile([128, HC * HD], BF16, tag=f"v4_{g}",
                             name=f"v4_{g}") for g in range(S // 128)]

            rs_in = [dram.tile([TS, MODEL], BF16, name=f"rsin{s}")
                     for s in range(NS)]
            rs_out = [dram.tile([128, MODEL], BF16, name=f"rsout{s}")
                      for s in range(NS)]
            for s in range(NS):
                cols = slice(TS * s, TS * (s + 1))
                # prefetch next span's inputs (bufs=2 so no queue blocking)
                if s + 1 < NS:
                    if s >= 1:
                        load_x8(s + 1)
                    load_xl(s + 1)
                if s == 0:
                    nc.sync.dma_start(out=tri_sb[:], in_=tri.ap())
                    nc.sync.dma_start(out=wo_sb[:], in_=wo.ap())
                    if not nob:
                        nc.sync.dma_start(out=bob_sb[:], in_=bob.ap())
                x8, xl = x8t[s], xlt[s]

                # ---- lq down-projection (fp8 DR) ----
                l2q = wpool.tile([128, LJ, TS], FP8, tag="l2q", bufs=2,
                                 name=f"l2q_{s}")
                for j in range(LJ):
                    ps = pspool.tile([128, TS], F32, tag="psL", bufs=2,
                                     name=f"pslq{s}{j}")
                    for p in range(NM // 2):
                        nc.tensor.matmul(
                            ps[:], wlq_sb[:, NM * j + 2 * p:NM * j + 2 * p + 2, :],
                            x8[:, 2 * p:2 * p + 2, :],
                            start=(p == 0), stop=(p == NM // 2 - 1),
                            perf_mode=PM.DoubleRow)
                    nc.scalar.activation(
                        l2q[:, j, :], ps[:], AF.Identity,
                        bias=bcon_sb[:, BLQ + j:BLQ + j + 1],
                        scale=SL / (SX * SW))

                # ---- pos_k down-projection + rope (fp8 DR) ----
                pspk = pspool.tile([128, TS], F32, tag="psL", bufs=2,
                                   name=f"pspk{s}")
                for p in range(NM // 2):
                    nc.tensor.matmul(
                        pspk[:], wpk_sb[:, 2 * p:2 * p + 2, :],
                        x8[:, 2 * p:2 * p + 2, :],
                        start=(p == 0), stop=(p == NM // 2 - 1),
                        perf_mode=PM.DoubleRow)
                t3p = wpool.tile([PHD, TS], F32, tag="pk34", bufs=2,
                                 name=f"pk3{s}")
                t4p = wpool.tile([PHD, TS], F32, tag="pk34", bufs=2,
                                 name=f"pk4{s}")
                nc.vector.scalar_tensor_tensor(
                    t3p[:], pspk[0:PHD, :], bcon_sb[0:PHD, BPK:BPK + 1],
                    scp_sb[0:PHD, cols], OP.add, OP.mult)
                nc.vector.scalar_tensor_tensor(
                    t4p[:], pspk[PHD:128, :], bcon_sb[PHD:128, BPK + 1:BPK + 2],
                    scp_sb[PHD:128, cols], OP.add, OP.mult)
                for h in range(HC):
                    nc.vector.tensor_tensor(
                        kTz[h][0:PHD, 1, cols], t3p[:], t4p[:], OP.add)

                # ---- fused k (fp8 DR) ----
                for h in range(HC):
                    ps = pspool.tile([128, TS], F32, tag="psL", bufs=2,
                                     name=f"pskf{s}{h}")
                    for p in range(NM // 2):
                        nc.tensor.matmul(
                            ps[:], wkf_sb[:, NM * h + 2 * p:NM * h + 2 * p + 2, :],
                            x8[:, 2 * p:2 * p + 2, :],
                            start=(p == 0), stop=(p == NM // 2 - 1),
                            perf_mode=PM.DoubleRow)
                    nc.scalar.activation(
                        kTz[h][:, 0, cols], ps[:], AF.Identity,
                        bias=bcon_sb[:, BKT + h:BKT + h + 1],
                        scale=SK / (SX * SW))

                # ---- lv down-projection (2-term fp8 residual split:
                # x_hi@w_hi + x_hi@w_lo + x_lo@w_hi, one psum chain) ----
                lv2 = wpool.tile([128, LJ, TS], BF16, tag="lv2", bufs=2,
                                 name=f"lv2_{s}")
                for j in range(LJ):
                    ps = pspool.tile([128, TS], F32, tag="psL", bufs=2,
                                     name=f"pslv{s}{j}")
                    for ci, (wt, xt) in enumerate(
                            ((wlvh_sb, x8), (wlvl_sb, x8), (wlvh_sb, xl))):
                        for p in range(NM // 2):
                            nc.tensor.matmul(
                                ps[:],
                                wt[:, NM * j + 2 * p:NM * j + 2 * p + 2, :],
                                xt[:, 2 * p:2 * p + 2, :],
                                start=(ci == 0 and p == 0),
                                stop=(ci == 2 and p == NM // 2 - 1),
                                perf_mode=PM.DoubleRow)
                    nc.vector.tensor_scalar(
                        lv2[:, j, :], ps[:], 1.0 / (SX * SW),
                        bcon_sb[:, 20 + j:20 + j + 1], OP.mult, OP.add)

                # ---- q up-projection (fp8 DR) ----
                qTz = [wpool.tile([128, 2, TS], FP8, tag=f"qTz{h}", bufs=3,
                                  name=f"qTz{h}_{s}") for h in range(HC)]
                for h in range(HC):
                    # fp8 garbage can decode as NaN; NaN*0 poisons the
                    # DoubleRow pos-pad product, so zero the pad rows
                    nc.vector.memset(qTz[h][PHD:128, 1, :], 0.0)
                for h in range(HC):
                    ps = pspool.tile([128, TS], F32, tag="psL", bufs=2,
                                     name=f"psq{s}{h}")
                    for p in range(LJ // 2):
                        nc.tensor.matmul(
                            ps[:],
                            wq_sb[:, 2 * p:2 * p + 2, 128 * h:128 * (h + 1)],
                            l2q[:, 2 * p:2 * p + 2, :],
                            start=(p == 0), stop=(p == LJ // 2 - 1),
                            perf_mode=PM.DoubleRow)
                    nc.scalar.activation(
                        qTz[h][:, 0, :], ps[:], AF.Identity,
                        bias=bcon_sb[:, BQT + h:BQT + h + 1],
                        scale=SQ / (SL * SW))

                # ---- q_pos up-projection + rope (fp8 DR) ----
                for p2 in range(2):
                    psr = pspool.tile([128, TS], F32, tag="psL", bufs=2,
                                      name=f"psqr{s}{p2}")
                    pso = pspool.tile([128, TS], F32, tag="psL", bufs=2,
                                      name=f"psqo{s}{p2}")
                    for p in range(LJ // 2):
                        nc.tensor.matmul(
                            psr[:],
                            wqp_sb[:, 2 * p:2 * p + 2,
                                   256 * p2:256 * p2 + 128],
                            l2q[:, 2 * p:2 * p + 2, :],
                            start=(p == 0), stop=(p == LJ // 2 - 1),
                            perf_mode=PM.DoubleRow)
                    for p in range(LJ // 2):
                        nc.tensor.matmul(
                            pso[:],
                            wqp_sb[:, 2 * p:2 * p + 2,
                                   256 * p2 + 128:256 * p2 + 256],
                            l2q[:, 2 * p:2 * p + 2, :],
                            start=(p == 0), stop=(p == LJ // 2 - 1),
                            perf_mode=PM.DoubleRow)
                    t3 = wpool.tile([128, TS], F32, tag="qp34", bufs=2,
                                    name=f"qp3{s}{p2}")
                    t4 = wpool.tile([128, TS], F32, tag="qp34", bufs=2,
                                    name=f"qp4{s}{p2}")
                    nc.vector.scalar_tensor_tensor(
                        t3[:], psr[:], bcon_sb[:, BQP + 2 * p2:
                                               BQP + 2 * p2 + 1],
                        sc2_sb[:, cols], OP.add, OP.mult)
                    nc.vector.scalar_tensor_tensor(
                        t4[:], pso[:], bcon_sb[:, BQP + 2 * p2 + 1:
                                               BQP + 2 * p2 + 2],
                        sc2_sb[:, S + TS * s:S + TS * (s + 1)],
                        OP.add, OP.mult)
                    for idx in range(2):
                        nc.vector.tensor_tensor(
                            qTz[2 * p2 + idx][0:PHD, 1, :],
                            t3[PHD * idx:PHD * (idx + 1), :],
                            t4[PHD * idx:PHD * (idx + 1), :], OP.add)

                # ---- v up-projection (bf16, all 4 heads per matmul) ----
                for tt in range(TS // 128):
                    psv = pspool.tile([128, 512], F32, tag="psX", bufs=2,
                                      name=f"psv{s}{tt}")
                    for j in range(LJ):
                        nc.tensor.matmul(
                            psv[:], lv2[:, j, 128 * tt:128 * (tt + 1)],
                            wvp_sb[:, j, :],
                            start=(j == 0), stop=(j == LJ - 1))
                    nc.any.tensor_copy(v4[4 * s + tt][:], psv[:])

                # ---- attention for this span ----
                attnT = [wpool.tile([128, TS], BF16, tag=f"at{h}", bufs=3,
                                    name=f"at{h}_{s}") for h in range(HC)]
                for h in range(HC):
                    ps_at = pspool.tile([128, TS], F32, tag="psA", bufs=2,
                                        name=f"psat{s}{h}")
                    ps_sumf = pspool.tile([128, TS], F32, tag="psX", bufs=2,
                                          name=f"pssum{s}{h}")

                    tmax = 4 * s + 3
                    for t in range(tmax + 1):
                        off = 128 * t - TS * s
                        qlo = max(0, off)
                        kc = 128 * t
                        qs = slice(qlo, TS)
                        ps_sc = pspool.tile([128, TS], F32, tag="psC",
                                            bufs=2, name=f"pssc{s}{h}{t}")
                        nc.tensor.matmul(
                            ps_sc[:, qs], kTz[h][:, :, kc:kc + 128],
                            qTz[h][:, :, qs],
                            start=True, stop=True, perf_mode=PM.DoubleRow)
                        pt = wpool.tile([128, TS], BF16, tag="pt", bufs=6,
                                        name=f"pt{s}{h}{t}")
                        nc.scalar.activation(pt[:, qs], ps_sc[:, qs],
                                             AF.Exp, scale=EXP_SCALE)
                        if off >= 0:
                            nc.any.tensor_tensor(
                                pt[:, qlo:qlo + 128],
                                pt[:, qlo:qlo + 128], tri_sb[:], OP.mult)
                        nc.tensor.matmul(
                            ps_at[:, qs], v4[t][:, HD * h:HD * (h + 1)],
                            pt[:, qs], start=(t == 0), stop=(t == tmax))
                        nc.tensor.matmul(
                            ps_sumf[0:1, qs], ones_col[:], pt[:, qs],
                            start=(t == 0), stop=(t == tmax))
                    recf = wpool.tile([1, TS], F32, tag="recf", bufs=2,
                                      name=f"recf{s}{h}")
                    nc.vector.reciprocal(recf[:], ps_sumf[0:1, :])
                    recb = wpool.tile([1, TS], BF16, tag="recb", bufs=2,
                                      name=f"recb{s}{h}")
                    nc.vector.tensor_copy(recb[:], recf[:])
                    ps_rb = pspool.tile([128, TS], F32, tag="psX", bufs=2,
                                        name=f"psrb{s}{h}")
                    nc.tensor.matmul(ps_rb[:], ones_row[:], recb[:],
                                     start=True, stop=True)
                    rb_sb = wpool.tile([128, TS], BF16, tag="rbsb", bufs=2,
                                       name=f"rbsb{s}{h}")
                    nc.any.tensor_copy(rb_sb[:], ps_rb[:])
                    nc.vector.tensor_tensor(
                        attnT[h][:], ps_at[:], rb_sb[:], OP.mult)

                # ---- partial o_proj for this span + ReduceScatter ----
                for tt in range(TS // 128):
                    for oc in range(4):
                        ps_d = pspool.tile([128, TS], F32, tag="psX", bufs=2,
                                           name=f"psd{s}{tt}{oc}")
                        for h in range(HC):
                            nc.tensor.matmul(
                                ps_d[:],
                                attnT[h][:, 128 * tt:128 * (tt + 1)],
                                wo_sb[:, 4 * h + oc, :],
                                start=(h == 0), stop=(h == HC - 1))
                        st = wpool.tile([128, TS], BF16, tag="st", bufs=4,
                                        name=f"st{s}{tt}{oc}")
                        if nob:
                            nc.any.tensor_copy(st[:], ps_d[:])
                        else:
                            nc.vector.tensor_tensor(
                                st[:], ps_d[:],
                                bob_sb[:, TS * oc:TS * (oc + 1)], OP.add)
                        nc.sync.dma_start(
                            out=rs_in[s][128 * tt:128 * (tt + 1),
                                         TS * oc:TS * (oc + 1)],
                            in_=st[:])
                nc.gpsimd.collective_compute(
                    "ReduceScatter", OP.add,
                    ins=[rs_in[s].opt()], outs=[rs_out[s].opt()],
                    replica_groups=groups)
                # DRAM->DRAM copies are slow in one shot; bounce via SBUF
                ob = wpool.tile([128, MODEL], BF16, tag="ob", bufs=1,
                                name=f"ob{s}")
                nc.sync.dma_start(out=ob[:, 0:1024], in_=rs_out[s][:, 0:1024])
                nc.scalar.dma_start(out=ob[:, 1024:2048],
                                    in_=rs_out[s][:, 1024:2048])
                nc.sync.dma_start(
                    out=out_sh.ap()[128 * s:128 * (s + 1), 0:1024],
                    in_=ob[:, 0:1024])
                nc.scalar.dma_start(
                    out=out_sh.ap()[128 * s:128 * (s + 1), 1024:2048],
                    in_=ob[:, 1024:2048])
    nc.compile()
    return nc


def _host_prep(inputs):
    x = np.asarray(inputs["x"], np.float32)
    w_qkv, b_qkv = np.asarray(inputs["w_qkv"], np.float32), \
        np.asarray(inputs["b_qkv"], np.float32)
    w_qup, b_qup = np.asarray(inputs["w_qup"], np.float32), \
        np.asarray(inputs["b_qup"], np.float32)
    w_kup, b_kup = np.asarray(inputs["w_kup"], np.float32), \
        np.asarray(inputs["b_kup"], np.float32)
    w_vup, b_vup = np.asarray(inputs["w_vup"], np.float32), \
        np.asarray(inputs["b_vup"], np.float32)
    w_qpos, b_qpos = np.asarray(inputs["w_qpos"], np.float32), \
        np.asarray(inputs["b_qpos"], np.float32)
    w_kpos, b_kpos = np.asarray(inputs["w_kpos"], np.float32), \
        np.asarray(inputs["b_kpos"], np.float32)
    w_o, b_o = np.asarray(inputs["w_o"], np.float32), \
        np.asarray(inputs["b_o"], np.float32)

    x_flat = x.reshape(B * S, MODEL)

    # rope tables (position within sequence; same for both batches)
    inv_freq = 1.0 / (THETA ** (np.arange(0, PHD, 2, dtype=np.float32) / PHD))
    pos = np.arange(S, dtype=np.float32)
    freqs = np.outer(pos, inv_freq)
    emb = np.concatenate([freqs, freqs], -1)            # [S, 64]
    cos = np.cos(emb).astype(np.float32)
    sin = np.sin(emb).astype(np.float32)
    sin_signed = np.concatenate([-sin[:, :32], sin[:, 32:]], -1)
    # stacked for 2 heads; pre-scaled by SQ/(SL*SW) (== SK/(SX*SW))
    tscale = SQ / (SL * SW)
    cosT = np.concatenate([cos, cos], 1).T * tscale     # [128, S]
    sinT = np.concatenate([sin_signed, sin_signed], 1).T * tscale
    sc2 = np.concatenate([cosT, sinT], 1).astype(BF)    # [128, 2S]
    # posk table: rows 0:64 cos, rows 64:128 sin_signed (partition-aligned
    # with the raw/rot halves of the posk psum)
    scp = np.concatenate([cosT[0:PHD], sinT[0:PHD]], 0).astype(BF)

    tri_m = np.triu(np.ones((128, 128), np.float32)).astype(BF)

    # b_vup flows through o_proj exactly: attn rows sum p to 1
    bo_eff = b_o + b_vup @ w_o
    bob = np.tile((bo_eff / G).reshape(1, MODEL), (128, 1)).astype(BF)

    def pack_kx(w2, scale, dtype):
        # [2048, C] -> [128, (C//128)*NM, 128]: per out-tile, K-chunk-major
        C = w2.shape[1]
        r = w2.reshape(NM, 128, C // 128, 128).transpose(1, 2, 0, 3)
        return np.ascontiguousarray(
            r.reshape(128, (C // 128) * NM * 128) * scale).astype(dtype)

    def pack_xt(x2, scale, dtype):
        n = x2.shape[0]
        return np.ascontiguousarray(
            x2.reshape(n // TS, TS, NM, 128).transpose(3, 0, 2, 1)
            .reshape(128, (n // TS) * NM * TS) * scale).astype(dtype)

    wkf_full = w_qkv[:, 512:1024] @ w_kup               # [2048, 2048]
    bkf_full = b_qkv[512:1024] @ w_kup + b_kup          # [2048]

    in_maps = []
    for c in range(NC):
        w = c % G
        h0 = HC * w
        cm = slice(HD * h0, HD * (h0 + HC))             # 4-head main cols
        cp = slice(PHD * h0, PHD * (h0 + HC))           # 4-head pos cols

        xsc = pack_xt(x_flat[S * (c // G):S * (c // G + 1)], SX,
                      np.float32)
        xb8_l = xsc.astype(F8)
        xlo_l = (xsc - xb8_l.astype(np.float32)).astype(F8)

        wlq_l = pack_kx(w_qkv[:, 0:512], SW, F8)
        wv64 = pack_kx(w_qkv[:, 1024:1536], SW, np.float32)
        wlvh_l = wv64.astype(F8)
        wlvl_l = (wv64 - wlvh_l.astype(np.float32)).astype(F8)
        wpk_l = pack_kx(
            np.concatenate([w_kpos, w_kpos[:, _ROT]], 1), SW, F8)
        wkf_l = pack_kx(wkf_full[:, cm], SW, F8)

        # q up: [512, 512] -> [128, LJ, HC*128]
        wq = w_qup[:, cm]
        wq_l = np.ascontiguousarray(
            wq.reshape(LJ, 128, HC * 128).transpose(1, 0, 2)
            .reshape(128, LJ * HC * 128) * SW).astype(F8)
        # qpos up: cols (p2, rr, 128): per pack p2: raw 128 (2 heads x 64),
        # then rot 128
        wp = w_qpos[:, cp]                               # [512, 256]
        wpr = np.concatenate(
            [wp[:, PHD * i:PHD * (i + 1)][:, _ROT] for i in range(HC)], 1)
        qp_cols = []
        for p2 in range(2):
            qp_cols.append(wp[:, 128 * p2:128 * (p2 + 1)])
            qp_cols.append(wpr[:, 128 * p2:128 * (p2 + 1)])
        wqp = np.concatenate(qp_cols, 1)                 # [512, 512]
        wqp_l = np.ascontiguousarray(
            wqp.reshape(LJ, 128, 512).transpose(1, 0, 2)
            .reshape(128, LJ * 512) * SW).astype(F8)
        # v up: [512, 512] -> [128, LJ, 512]
        wv = w_vup[:, cm]
        wvp_l = np.ascontiguousarray(
            wv.reshape(LJ, 128, 512).transpose(1, 0, 2)
            .reshape(128, LJ * 512)).astype(BF)
        # o_proj rows for this core's heads: [128, (h, oc), 512]
        wol = np.ascontiguousarray(
            w_o[cm, :].reshape(HC, 128, 4, TS).transpose(1, 0, 2, 3)
            .reshape(128, HC * 4 * TS)).astype(BF)

        bc = np.zeros((128, 32), np.float32)
        for j in range(LJ):
            bc[:, 0 + j] = b_qkv[128 * j:128 * (j + 1)] * SL
            bc[:, 20 + j] = b_qkv[1024 + 128 * j:1024 + 128 * (j + 1)]
        bc[0:PHD, 4] = b_kpos * (SX * SW)
        bc[PHD:128, 5] = b_kpos[_ROT] * (SX * SW)
        for h in range(HC):
            bc[:, 8 + h] = bkf_full[cm][128 * h:128 * (h + 1)] * SK
            bc[:, 12 + h] = b_qup[cm][128 * h:128 * (h + 1)] * SQ
        for p2 in range(2):
            bq2 = np.concatenate(
                [b_qpos[PHD * (h0 + 2 * p2 + i):PHD * (h0 + 2 * p2 + i + 1)]
                 for i in range(2)])                     # [128]
            bc[:, 16 + 2 * p2] = bq2 * (SL * SW)
            bc[:, 16 + 2 * p2 + 1] = np.concatenate(
                [bq2[0:PHD][_ROT], bq2[PHD:128][_ROT]]) * (SL * SW)

        m = {"xb8": xb8_l, "xlo8": xlo_l, "wlq8": wlq_l, "wpk8": wpk_l,
             "wkf8": wkf_l, "wlvh8": wlvh_l, "wlvl8": wlvl_l,
             "wq8": wq_l, "wqp8": wqp_l,
             "wvp": wvp_l, "wo": wol, "sc2": sc2, "scp": scp, "bcon": bc,
             "bob": bob, "tri": tri_m}
        in_maps.append(m)
    return in_maps


def kernel(**inputs) -> np.ndarray:
    nob = (not np.any(np.asarray(inputs["b_o"]))
           and not np.any(np.asarray(inputs["b_vup"])))
    key = f"nc{int(nob)}"
    if key not in _CACHE:
        _CACHE[key] = _build(nob)
    _CACHE["nc"] = _CACHE[key]
    nc = _CACHE[key]
    in_maps = _host_prep({k: np.asarray(v) for k, v in inputs.items()})
    res = run_bass_kernel_spmd(nc, in_maps, list(range(NC))).results
    out = np.zeros((B, S, MODEL), np.float32)
    for c in range(NC):
        w = c % G
        o = res[c]["out_sh"].astype(np.float32)          # [NS*128, MODEL]
        for s in range(NS):
            out[c // G, TS * s + 128 * w:TS * s + 128 * (w + 1), :] = \
                o[128 * s:128 * (s + 1), :]
    return out


# revision 40
# speedup vs baseline: 1.1378x; 1.1378x over previous
"""Multi-head latent attention (MLA) Trainium2 kernel, 8-core SPMD, v2.

Sharding: 2 batch-groups of 4 cores (cores 0-3 = batch 0, 4-7 = batch 1).
Within a group, core w owns heads {4w..4w+3} and, at the end, token rows
{128w+128j} of each span's ReduceScatter output.

Design (fully span-pipelined, no mid-kernel collective):
  - Replicate the latent down-projections per core over its whole batch:
    the fp8 ones are nearly free with DoubleRow, and the bf16 lv fills
    engine gaps in the attention phase; an AllGather variant measured
    slower (the transfer can't hide behind the cheap fp8 linear phase).
  - fp8e4m3 + DoubleRow matmuls (2 K-chunks per instruction) for the whole
    q/k/score path: lq down-proj, pos_k down-proj, fused k (x @ (Wkv_k@Wkup)),
    q up-proj, q_pos up-proj, and the score matmuls (main+positional parts
    paired into one DoubleRow instruction with zero-padded pos rows).
  - bf16 for the v path (lv down-proj, v up-proj), attention p@v, and o_proj:
    fp8 there fails the 2e-2 error budget (measured on the real inputs).
  - All biases folded host-side; b_vup routes through o_proj exactly as
    (b_vup @ w_o) added to b_o since softmax rows sum to 1.
  - Per-span partial o_proj + per-span ReduceScatter (4 small collectives
    that overlap compute; only the last one's ~28us is exposed).
RoPE via pre-permuted weight copies: rope(u) = u*cos + perm(u)*sin_signed.
Softmax is max-free (scores bounded): p = exp(s*scale), denominators via
ones-column matmul, normalization after the v-matmul.
"""
import numpy as np
import ml_dtypes

import concourse.bacc as bacc
import concourse.mybir as mybir
import concourse.tile as tile
from concourse.bass_utils import run_bass_kernel_spmd
from concourse.tile import add_dep_helper


def _dep(a, b, reason):
    add_dep_helper(getattr(a, 'ins', a), getattr(b, 'ins', b), sync=False,
                   reason=reason)

F32 = mybir.dt.float32
BF16 = mybir.dt.bfloat16
FP8 = mybir.dt.float8e4
AF = mybir.ActivationFunctionType
OP = mybir.AluOpType
PM = mybir.MatmulPerfMode
BF = ml_dtypes.bfloat16
F8 = ml_dtypes.float8_e4m3

MODEL = 2048
LATENT = 512
NH = 16
HD = 128
PHD = 64
THETA = 50000.0
B = 2
S = 2048
NC = 8
G = 4              # cores per batch-group
TS = 512           # tokens per span
NS = S // TS       # 4 spans
HC = NH // G       # 4 heads per core
NM = MODEL // 128  # 16 K-chunks of the model dim
LJ = LATENT // 128  # 4 K-chunks of the latent dim
SCALE = 1.0 / float(np.sqrt(HD + PHD))

# fp8 scale factors
SX = 16.0   # x
SW = 64.0   # weights
SL = 16.0   # lq latents
SQ = 16.0   # q-side activations
SK = 16.0   # k-side activations
EXP_SCALE = SCALE / (SQ * SK)

_ROT = np.r_[32:64, 0:32]

_CACHE = {}


def _build(nob):
    nc = bacc.Bacc("TRN2", target_bir_lowering=False, debug=False,
                   num_devices=NC)

    xb8 = nc.dram_tensor("xb8", [128, NS * NM * TS], FP8,
                         kind="ExternalInput")
    xlo8 = nc.dram_tensor("xlo8", [128, NS * NM * TS], FP8,
                          kind="ExternalInput")
    wlq8 = nc.dram_tensor("wlq8", [128, LJ * NM * 128], FP8,
                          kind="ExternalInput")
    wpk8 = nc.dram_tensor("wpk8", [128, NM * 128], FP8, kind="ExternalInput")
    wkf8 = nc.dram_tensor("wkf8", [128, HC * NM * 128], FP8,
                          kind="ExternalInput")
    wlvh8 = nc.dram_tensor("wlvh8", [128, LJ * NM * 128], FP8,
                           kind="ExternalInput")
    wlvl8 = nc.dram_tensor("wlvl8", [128, LJ * NM * 128], FP8,
                           kind="ExternalInput")
    wq8 = nc.dram_tensor("wq8", [128, LJ * HC * 128], FP8,
                         kind="ExternalInput")
    wqp8 = nc.dram_tensor("wqp8", [128, LJ * 512], FP8, kind="ExternalInput")
    wvp = nc.dram_tensor("wvp", [128, LJ * 512], BF16, kind="ExternalInput")
    wo = nc.dram_tensor("wo", [128, HC * 4 * TS], BF16, kind="ExternalInput")
    sc2 = nc.dram_tensor("sc2", [128, 2 * S], BF16, kind="ExternalInput")
    scp = nc.dram_tensor("scp", [128, S], BF16, kind="ExternalInput")
    bcon = nc.dram_tensor("bcon", [128, 32], F32, kind="ExternalInput")
    bob = nc.dram_tensor("bob", [128, MODEL], BF16, kind="ExternalInput")
    tri = nc.dram_tensor("tri", [128, 128], BF16, kind="ExternalInput")
    out_sh = nc.dram_tensor("out_sh", [NS * 128, MODEL], BF16,
                            kind="ExternalOutput")

    # bcon column map: 0:4 lq bias (per j), 4 posk raw, 5 posk rot,
    # 8:12 kT bias (per h), 12:16 qT bias (per h), 16:20 qpos (p,rr)
    BLQ, BPK, BKT, BQT, BQP = 0, 4, 8, 12, 16

    groups = [[0, 1, 2, 3], [4, 5, 6, 7]]

    with tile.TileContext(nc) as tc:
        with (
            tc.tile_pool(name="const", bufs=1) as cpool,
            tc.tile_pool(name="work", bufs=1) as wpool,
            tc.tile_pool(name="psum", bufs=1, space="PSUM") as pspool,
            tc.tile_pool(name="dram", bufs=1, space="DRAM") as dram,
        ):
            # ---- constants + span-0 inputs, DMA-ordered by first use ----
            bcon_sb = cpool.tile([128, 32], F32, tag="bcon")
            nc.sync.dma_start(out=bcon_sb[:], in_=bcon.ap())
            wlq_sb = cpool.tile([128, LJ * NM, 128], FP8, tag="wlq")
            nc.sync.dma_start(out=wlq_sb[:, 0:NM, :],
                              in_=wlq8.ap()[:, 0:NM * 128])
            x8t = [wpool.tile([128, NM, TS], FP8, tag="x8", bufs=2,
                              name=f"x8_{s}") for s in range(NS)]
            xlt = [wpool.tile([128, NM, TS], FP8, tag="xl", bufs=2,
                              name=f"xl_{s}") for s in range(NS)]

            def load_x8(s):
                for ch in range(4):
                    nc.sync.dma_start(
                        out=x8t[s][:, 4 * ch:4 * (ch + 1), :],
                        in_=xb8.ap()[:, NM * TS * s + 4 * TS * ch:
                                     NM * TS * s + 4 * TS * (ch + 1)])

            def load_xl(s):
                for ch in range(4):
                    nc.sync.dma_start(
                        out=xlt[s][:, 4 * ch:4 * (ch + 1), :],
                        in_=xlo8.ap()[:, NM * TS * s + 4 * TS * ch:
                                      NM * TS * s + 4 * TS * (ch + 1)])

            for ch in range(4):
                nc.sync.dma_start(
                    out=x8t[0][:, 4 * ch:4 * (ch + 1), :],
                    in_=xb8.ap()[:, 4 * TS * ch:4 * TS * (ch + 1)])
            nc.sync.dma_start(out=wlq_sb[:, NM:LJ * NM, :],
                              in_=wlq8.ap()[:, NM * 128:LJ * NM * 128])
            wkf_sb = cpool.tile([128, HC * NM, 128], FP8, tag="wkf")
            nc.sync.dma_start(out=wkf_sb[:], in_=wkf8.ap())
            wpk_sb = cpool.tile([128, NM, 128], FP8, tag="wpk")
            nc.sync.dma_start(out=wpk_sb[:], in_=wpk8.ap())
            scp_sb = cpool.tile([128, S], BF16, tag="scp")
            nc.sync.dma_start(out=scp_sb[:], in_=scp.ap())
            for ch in range(4):
                nc.sync.dma_start(
                    out=xlt[0][:, 4 * ch:4 * (ch + 1), :],
                    in_=xlo8.ap()[:, 4 * TS * ch:4 * TS * (ch + 1)])
            wlvh_sb = cpool.tile([128, LJ * NM, 128], FP8, tag="wlvh")
            nc.sync.dma_start(out=wlvh_sb[:], in_=wlvh8.ap())
            wlvl_sb = cpool.tile([128, LJ * NM, 128], FP8, tag="wlvl")
            nc.sync.dma_start(out=wlvl_sb[:], in_=wlvl8.ap())
            load_x8(1)
            wq_sb = cpool.tile([128, LJ, HC * 128], FP8, tag="wq")
            nc.sync.dma_start(out=wq_sb[:], in_=wq8.ap())
            wqp_sb = cpool.tile([128, LJ, 512], FP8, tag="wqp")
            nc.sync.dma_start(out=wqp_sb[:], in_=wqp8.ap())
            sc2_sb = cpool.tile([128, 2 * S], BF16, tag="sc2")
            nc.sync.dma_start(out=sc2_sb[:], in_=sc2.ap())
            wvp_sb = cpool.tile([128, LJ, 512], BF16, tag="wvp")
            nc.sync.dma_start(out=wvp_sb[:], in_=wvp.ap())
            tri_sb = cpool.tile([128, 128], BF16, tag="tri")
            wo_sb = cpool.tile([128, HC * 4, TS], BF16, tag="wo")
            bob_sb = cpool.tile([128, MODEL], BF16, tag="bob")
            ones_col = cpool.tile([128, 1], BF16, tag="onesc")
            nc.vector.memset(ones_col[:], 1.0)
            ones_row = cpool.tile([1, 128], BF16, tag="onesr")
            nc.vector.memset(ones_row[:], 1.0)

            # full-sequence k tensors: [feat 128, i(main/pos), S]
            kTz = [cpool.tile([128, 2, S], FP8, tag=f"kTz{h}",
                              name=f"kTz{h}") for h in range(HC)]
            for h in range(HC):
                # zero the unused pos rows once; q-side pos rows are garbage
                # and get multiplied by these zeros
                nc.vector.memset(kTz[h][64:128, 1, :], 0.0)
            v4 = [cpool.tile([128, HC * HD], BF16, tag=f"v4_{g}",
                             name=f"v4_{g}") for g in range(S // 128)]

            rs_in = [dram.tile([TS, MODEL], BF16, name=f"rsin{s}")
                     for s in range(NS)]
            rs_out = [dram.tile([128, MODEL], BF16, name=f"rsout{s}")
                      for s in range(NS)]
            for s in range(NS):
                cols = slice(TS * s, TS * (s + 1))
                # prefetch next span's inputs (bufs=2 so no queue blocking)
                if s + 1 < NS:
                    if s >= 1:
                        load_x8(s + 1)
                    load_xl(s + 1)
                if s == 0:
                    nc.sync.dma_start(out=tri_sb[:], in_=tri.ap())
                    nc.sync.dma_start(out=wo_sb[:], in_=wo.ap())
                    if not nob:
                        nc.sync.dma_start(out=bob_sb[:], in_=bob.ap())
                x8, xl = x8t[s], xlt[s]

                # ---- lq down-projection (fp8 DR) ----
                l2q = wpool.tile([128, LJ, TS], FP8, tag="l2q", bufs=2,
                                 name=f"l2q_{s}")
                for j in range(LJ):
                    ps = pspool.tile([128, TS], F32, tag="psL", bufs=2,
                                     name=f"pslq{s}{j}")
                    for p in range(NM // 2):
                        nc.tensor.matmul(
                            ps[:], wlq_sb[:, NM * j + 2 * p:NM * j + 2 * p + 2, :],
                            x8[:, 2 * p:2 * p + 2, :],
                            start=(p == 0), stop=(p == NM // 2 - 1),
                            perf_mode=PM.DoubleRow)
                    nc.scalar.activation(
                        l2q[:, j, :], ps[:], AF.Identity,
                        bias=bcon_sb[:, BLQ + j:BLQ + j + 1],
                        scale=SL / (SX * SW))

                # ---- pos_k down-projection + rope (fp8 DR) ----
                pspk = pspool.tile([128, TS], F32, tag="psL", bufs=2,
                                   name=f"pspk{s}")
                for p in range(NM // 2):
                    nc.tensor.matmul(
                        pspk[:], wpk_sb[:, 2 * p:2 * p + 2, :],
                        x8[:, 2 * p:2 * p + 2, :],
                        start=(p == 0), stop=(p == NM // 2 - 1),
                        perf_mode=PM.DoubleRow)
                t3p = wpool.tile([PHD, TS], F32, tag="pk34", bufs=2,
                                 name=f"pk3{s}")
                t4p = wpool.tile([PHD, TS], F32, tag="pk34", bufs=2,
                                 name=f"pk4{s}")
                nc.vector.scalar_tensor_tensor(
                    t3p[:], pspk[0:PHD, :], bcon_sb[0:PHD, BPK:BPK + 1],
                    scp_sb[0:PHD, cols], OP.add, OP.mult)
                nc.vector.scalar_tensor_tensor(
                    t4p[:], pspk[PHD:128, :], bcon_sb[PHD:128, BPK + 1:BPK + 2],
                    scp_sb[PHD:128, cols], OP.add, OP.mult)
                for h in range(HC):
                    nc.vector.tensor_tensor(
                        kTz[h][0:PHD, 1, cols], t3p[:], t4p[:], OP.add)

                # ---- fused k (fp8 DR) ----
                for h in range(HC):
                    ps = pspool.tile([128, TS], F32, tag="psL", bufs=2,
                                     name=f"pskf{s}{h}")
                    for p in range(NM // 2):
                        nc.tensor.matmul(
                            ps[:], wkf_sb[:, NM * h + 2 * p:NM * h + 2 * p + 2, :],
                            x8[:, 2 * p:2 * p + 2, :],
                            start=(p == 0), stop=(p == NM // 2 - 1),
                            perf_mode=PM.DoubleRow)
                    nc.scalar.activation(
                        kTz[h][:, 0, cols], ps[:], AF.Identity,
                        bias=bcon_sb[:, BKT + h:BKT + h + 1],
                        scale=SK / (SX * SW))

                # ---- lv down-projection (2-term fp8 residual split:
                # x_hi@w_hi + x_hi@w_lo + x_lo@w_hi, one psum chain) ----
                lv2 = wpool.tile([128, LJ, TS], BF16, tag="lv2", bufs=2,
                                 name=f"lv2_{s}")
                for j in range(LJ):
                    ps = pspool.tile([128, TS], F32, tag="psL", bufs=2,
                                     name=f"pslv{s}{j}")
                    for ci, (wt, xt) in enumerate(
                            ((wlvh_sb, x8), (wlvl_sb, x8), (wlvh_sb, xl))):
                        for p in range(NM // 2):
                            nc.tensor.matmul(
                                ps[:],
                                wt[:, NM * j + 2 * p:NM * j + 2 * p + 2, :],
                                xt[:, 2 * p:2 * p + 2, :],
                                start=(ci == 0 and p == 0),
                                stop=(ci == 2 and p == NM // 2 - 1),
                                perf_mode=PM.DoubleRow)
                    nc.vector.tensor_scalar(
                        lv2[:, j, :], ps[:], 1.0 / (SX * SW),
                        bcon_sb[:, 20 + j:20 + j + 1], OP.mult, OP.add)

                # ---- q up-projection (fp8 DR) ----
                qTz = [wpool.tile([128, 2, TS], FP8, tag=f"qTz{h}", bufs=3,
                                  name=f"qTz{h}_{s}") for h in range(HC)]
                for h in range(HC):
                    # fp8 garbage can decode as NaN; NaN*0 poisons the
                    # DoubleRow pos-pad product, so zero the pad rows
                    nc.vector.memset(qTz[h][PHD:128, 1, :], 0.0)
                for h in range(HC):
                    ps = pspool.tile([128, TS], F32, tag="psL", bufs=2,
                                     name=f"psq{s}{h}")
                    for p in range(LJ // 2):
                        nc.tensor.matmul(
                            ps[:],
                            wq_sb[:, 2 * p:2 * p + 2, 128 * h:128 * (h + 1)],
                            l2q[:, 2 * p:2 * p + 2, :],
                            start=(p == 0), stop=(p == LJ // 2 - 1),
                            perf_mode=PM.DoubleRow)
                    nc.scalar.activation(
                        qTz[h][:, 0, :], ps[:], AF.Identity,
                        bias=bcon_sb[:, BQT + h:BQT + h + 1],
                        scale=SQ / (SL * SW))

                # ---- q_pos up-projection + rope (fp8 DR) ----
                for p2 in range(2):
                    psr = pspool.tile([128, TS], F32, tag="psL", bufs=2,
                                      name=f"psqr{s}{p2}")
                    pso = pspool.tile([128, TS], F32, tag="psL", bufs=2,
                                      name=f"psqo{s}{p2}")
                    for p in range(LJ // 2):
                        nc.tensor.matmul(
                            psr[:],
                            wqp_sb[:, 2 * p:2 * p + 2,
                                   256 * p2:256 * p2 + 128],
                            l2q[:, 2 * p:2 * p + 2, :],
                            start=(p == 0), stop=(p == LJ // 2 - 1),
                            perf_mode=PM.DoubleRow)
                    for p in range(LJ // 2):
                        nc.tensor.matmul(
                            pso[:],
                            wqp_sb[:, 2 * p:2 * p + 2,
                                   256 * p2 + 128:256 * p2 + 256],
                            l2q[:, 2 * p:2 * p + 2, :],
                            start=(p == 0), stop=(p == LJ // 2 - 1),
                            perf_mode=PM.DoubleRow)
                    t3 = wpool.tile([128, TS], F32, tag="qp34", bufs=2,
                                    name=f"qp3{s}{p2}")
                    t4 = wpool.tile([128, TS], F32, tag="qp34", bufs=2,
                                    name=f"qp4{s}{p2}")
                    nc.vector.scalar_tensor_tensor(
                        t3[:], psr[:], bcon_sb[:, BQP + 2 * p2:
                                               BQP + 2 * p2 + 1],
                        sc2_sb[:, cols], OP.add, OP.mult)
                    nc.vector.scalar_tensor_tensor(
                        t4[:], pso[:], bcon_sb[:, BQP + 2 * p2 + 1:
                                               BQP + 2 * p2 + 2],
                        sc2_sb[:, S + TS * s:S + TS * (s + 1)],
                        OP.add, OP.mult)
                    for idx in range(2):
                        nc.vector.tensor_tensor(
                            qTz[2 * p2 + idx][0:PHD, 1, :],
                            t3[PHD * idx:PHD * (idx + 1), :],
                            t4[PHD * idx:PHD * (idx + 1), :], OP.add)

                # ---- v up-projection (bf16, all 4 heads per matmul) ----
                for tt in range(TS // 128):
                    psv = pspool.tile([128, 512], F32, tag="psX", bufs=2,
                                      name=f"psv{s}{tt}")
                    for j in range(LJ):
                        nc.tensor.matmul(
                            psv[:], lv2[:, j, 128 * tt:128 * (tt + 1)],
                            wvp_sb[:, j, :],
                            start=(j == 0), stop=(j == LJ - 1))
                    nc.any.tensor_copy(v4[4 * s + tt][:], psv[:])

                # ---- attention for this span ----
                attnT = [wpool.tile([128, TS], BF16, tag=f"at{h}", bufs=3,
                                    name=f"at{h}_{s}") for h in range(HC)]
                for h in range(HC):
                    ps_at = pspool.tile([128, TS], F32, tag="psA", bufs=2,
                                        name=f"psat{s}{h}")
                    ps_sumf = pspool.tile([128, TS], F32, tag="psX", bufs=2,
                                          name=f"pssum{s}{h}")

                    tmax = 4 * s + 3
                    for t in range(tmax + 1):
                        off = 128 * t - TS * s
                        qlo = max(0, off)
                        kc = 128 * t
                        qs = slice(qlo, TS)
                        ps_sc = pspool.tile([128, TS], F32, tag="psC",
                                            bufs=2, name=f"pssc{s}{h}{t}")
                        nc.tensor.matmul(
                            ps_sc[:, qs], kTz[h][:, :, kc:kc + 128],
                            qTz[h][:, :, qs],
                            start=True, stop=True, perf_mode=PM.DoubleRow)
                        pt = wpool.tile([128, TS], BF16, tag="pt", bufs=6,
                                        name=f"pt{s}{h}{t}")
                        nc.scalar.activation(pt[:, qs], ps_sc[:, qs],
                                             AF.Exp, scale=EXP_SCALE)
                        if off >= 0:
                            nc.any.tensor_tensor(
                                pt[:, qlo:qlo + 128],
                                pt[:, qlo:qlo + 128], tri_sb[:], OP.mult)
                        nc.tensor.matmul(
                            ps_at[:, qs], v4[t][:, HD * h:HD * (h + 1)],
                            pt[:, qs], start=(t == 0), stop=(t == tmax))
                        nc.tensor.matmul(
                            ps_sumf[0:1, qs], ones_col[:], pt[:, qs],
                            start=(t == 0), stop=(t == tmax))
                    recf = wpool.tile([1, TS], F32, tag="recf", bufs=2,
                                      name=f"recf{s}{h}")
                    nc.vector.reciprocal(recf[:], ps_sumf[0:1, :])
                    recb = wpool.tile([1, TS], BF16, tag="recb", bufs=2,
                                      name=f"recb{s}{h}")
                    nc.vector.tensor_copy(recb[:], recf[:])
                    ps_rb = pspool.tile([128, TS], F32, tag="psX", bufs=2,
                                        name=f"psrb{s}{h}")
                    nc.tensor.matmul(ps_rb[:], ones_row[:], recb[:],
                                     start=True, stop=True)
                    rb_sb = wpool.tile([128, TS], BF16, tag="rbsb", bufs=2,
                                       name=f"rbsb{s}{h}")
                    nc.any.tensor_copy(rb_sb[:], ps_rb[:])
                    nc.vector.tensor_tensor(
                        attnT[h][:], ps_at[:], rb_sb[:], OP.mult)

                # ---- partial o_proj for this span + ReduceScatter ----
                for tt in range(TS // 128):
                    for oc in range(4):
                        ps_d = pspool.tile([128, TS], F32, tag="psX", bufs=2,
                                           name=f"psd{s}{tt}{oc}")
                        for h in range(HC):
                            nc.tensor.matmul(
                                ps_d[:],
                                attnT[h][:, 128 * tt:128 * (tt + 1)],
                                wo_sb[:, 4 * h + oc, :],
                                start=(h == 0), stop=(h == HC - 1))
                        st = wpool.tile([128, TS], BF16, tag="st", bufs=4,
                                        name=f"st{s}{tt}{oc}")
                        if nob:
                            nc.any.tensor_copy(st[:], ps_d[:])
                        else:
                            nc.vector.tensor_tensor(
                                st[:], ps_d[:],
                                bob_sb[:, TS * oc:TS * (oc + 1)], OP.add)
                        nc.sync.dma_start(
                            out=rs_in[s][128 * tt:128 * (tt + 1),
                                         TS * oc:TS * (oc + 1)],
                            in_=st[:])
                nc.gpsimd.collective_compute(
                    "ReduceScatter", OP.add,
                    ins=[rs_in[s].opt()], outs=[rs_out[s].opt()],
                    replica_groups=groups)
                # DRAM->DRAM copies are slow in one shot; bounce via SBUF
                ob = wpool.tile([128, MODEL], BF16, tag="ob", bufs=1,
                                name=f"ob{s}")
                nc.sync.dma_start(out=ob[:], in_=rs_out[s][:])
                nc.sync.dma_start(
                    out=out_sh.ap()[128 * s:128 * (s + 1), :], in_=ob[:])
    nc.compile()
    return nc


def _host_prep(inputs):
    x = np.asarray(inputs["x"], np.float32)
    w_qkv, b_qkv = np.asarray(inputs["w_qkv"], np.float32), \
        np.asarray(inputs["b_qkv"], np.float32)
    w_qup, b_qup = np.asarray(inputs["w_qup"], np.float32), \
        np.asarray(inputs["b_qup"], np.float32)
    w_kup, b_kup = np.asarray(inputs["w_kup"], np.float32), \
        np.asarray(inputs["b_kup"], np.float32)
    w_vup, b_vup = np.asarray(inputs["w_vup"], np.float32), \
        np.asarray(inputs["b_vup"], np.float32)
    w_qpos, b_qpos = np.asarray(inputs["w_qpos"], np.float32), \
        np.asarray(inputs["b_qpos"], np.float32)
    w_kpos, b_kpos = np.asarray(inputs["w_kpos"], np.float32), \
        np.asarray(inputs["b_kpos"], np.float32)
    w_o, b_o = np.asarray(inputs["w_o"], np.float32), \
        np.asarray(inputs["b_o"], np.float32)

    x_flat = x.reshape(B * S, MODEL)

    # rope tables (position within sequence; same for both batches)
    inv_freq = 1.0 / (THETA ** (np.arange(0, PHD, 2, dtype=np.float32) / PHD))
    pos = np.arange(S, dtype=np.float32)
    freqs = np.outer(pos, inv_freq)
    emb = np.concatenate([freqs, freqs], -1)            # [S, 64]
    cos = np.cos(emb).astype(np.float32)
    sin = np.sin(emb).astype(np.float32)
    sin_signed = np.concatenate([-sin[:, :32], sin[:, 32:]], -1)
    # stacked for 2 heads; pre-scaled by SQ/(SL*SW) (== SK/(SX*SW))
    tscale = SQ / (SL * SW)
    cosT = np.concatenate([cos, cos], 1).T * tscale     # [128, S]
    sinT = np.concatenate([sin_signed, sin_signed], 1).T * tscale
    sc2 = np.concatenate([cosT, sinT], 1).astype(BF)    # [128, 2S]
    # posk table: rows 0:64 cos, rows 64:128 sin_signed (partition-aligned
    # with the raw/rot halves of the posk psum)
    scp = np.concatenate([cosT[0:PHD], sinT[0:PHD]], 0).astype(BF)

    tri_m = np.triu(np.ones((128, 128), np.float32)).astype(BF)

    # b_vup flows through o_proj exactly: attn rows sum p to 1
    bo_eff = b_o + b_vup @ w_o
    bob = np.tile((bo_eff / G).reshape(1, MODEL), (128, 1)).astype(BF)

    def pack_kx(w2, scale, dtype):
        # [2048, C] -> [128, (C//128)*NM, 128]: per out-tile, K-chunk-major
        C = w2.shape[1]
        r = w2.reshape(NM, 128, C // 128, 128).transpose(1, 2, 0, 3)
        return np.ascontiguousarray(
            r.reshape(128, (C // 128) * NM * 128) * scale).astype(dtype)

    def pack_xt(x2, scale, dtype):
        n = x2.shape[0]
        return np.ascontiguousarray(
            x2.reshape(n // TS, TS, NM, 128).transpose(3, 0, 2, 1)
            .reshape(128, (n // TS) * NM * TS) * scale).astype(dtype)

    wkf_full = w_qkv[:, 512:1024] @ w_kup               # [2048, 2048]
    bkf_full = b_qkv[512:1024] @ w_kup + b_kup          # [2048]

    in_maps = []
    for c in range(NC):
        w = c % G
        h0 = HC * w
        cm = slice(HD * h0, HD * (h0 + HC))             # 4-head main cols
        cp = slice(PHD * h0, PHD * (h0 + HC))           # 4-head pos cols

        xsc = pack_xt(x_flat[S * (c // G):S * (c // G + 1)], SX,
                      np.float32)
        xb8_l = xsc.astype(F8)
        xlo_l = (xsc - xb8_l.astype(np.float32)).astype(F8)

        wlq_l = pack_kx(w_qkv[:, 0:512], SW, F8)
        wv64 = pack_kx(w_qkv[:, 1024:1536], SW, np.float32)
        wlvh_l = wv64.astype(F8)
        wlvl_l = (wv64 - wlvh_l.astype(np.float32)).astype(F8)
        wpk_l = pack_kx(
            np.concatenate([w_kpos, w_kpos[:, _ROT]], 1), SW, F8)
        wkf_l = pack_kx(wkf_full[:, cm], SW, F8)

        # q up: [512, 512] -> [128, LJ, HC*128]
        wq = w_qup[:, cm]
        wq_l = np.ascontiguousarray(
            wq.reshape(LJ, 128, HC * 128).transpose(1, 0, 2)
            .reshape(128, LJ * HC * 128) * SW).astype(F8)
        # qpos up: cols (p2, rr, 128): per pack p2: raw 128 (2 heads x 64),
        # then rot 128
        wp = w_qpos[:, cp]                               # [512, 256]
        wpr = np.concatenate(
            [wp[:, PHD * i:PHD * (i + 1)][:, _ROT] for i in range(HC)], 1)
        qp_cols = []
        for p2 in range(2):
            qp_cols.append(wp[:, 128 * p2:128 * (p2 + 1)])
            qp_cols.append(wpr[:, 128 * p2:128 * (p2 + 1)])
        wqp = np.concatenate(qp_cols, 1)                 # [512, 512]
        wqp_l = np.ascontiguousarray(
            wqp.reshape(LJ, 128, 512).transpose(1, 0, 2)
            .reshape(128, LJ * 512) * SW).astype(F8)
        # v up: [512, 512] -> [128, LJ, 512]
        wv = w_vup[:, cm]
        wvp_l = np.ascontiguousarray(
            wv.reshape(LJ, 128, 512).transpose(1, 0, 2)
            .reshape(128, LJ * 512)).astype(BF)
        # o_proj rows for this core's heads: [128, (h, oc), 512]
        wol = np.ascontiguousarray(
            w_o[cm, :].reshape(HC, 128, 4, TS).transpose(1, 0, 2, 3)
            .reshape(128, HC * 4 * TS)).astype(BF)

        bc = np.zeros((128, 32), np.float32)
        for j in range(LJ):
            bc[:, 0 + j] = b_qkv[128 * j:128 * (j + 1)] * SL
            bc[:, 20 + j] = b_qkv[1024 + 128 * j:1024 + 128 * (j + 1)]
        bc[0:PHD, 4] = b_kpos * (SX * SW)
        bc[PHD:128, 5] = b_kpos[_ROT] * (SX * SW)
        for h in range(HC):
            bc[:, 8 + h] = bkf_full[cm][128 * h:128 * (h + 1)] * SK
            bc[:, 12 + h] = b_qup[cm][128 * h:128 * (h + 1)] * SQ
        for p2 in range(2):
            bq2 = np.concatenate(
                [b_qpos[PHD * (h0 + 2 * p2 + i):PHD * (h0 + 2 * p2 + i + 1)]
                 for i in range(2)])                     # [128]
            bc[:, 16 + 2 * p2] = bq2 * (SL * SW)
            bc[:, 16 + 2 * p2 + 1] = np.concatenate(
                [bq2[0:PHD][_ROT], bq2[PHD:128][_ROT]]) * (SL * SW)

        m = {"xb8": xb8_l, "xlo8": xlo_l, "wlq8": wlq_l, "wpk8": wpk_l,
             "wkf8": wkf_l, "wlvh8": wlvh_l, "wlvl8": wlvl_l,
             "wq8": wq_l, "wqp8": wqp_l,
             "wvp": wvp_l, "wo": wol, "sc2": sc2, "scp": scp, "bcon": bc,
             "bob": bob, "tri": tri_m}
        in_maps.append(m)
    return in_maps


def kernel(**inputs) -> np.ndarray:
    nob = (not np.any(np.asarray(inputs["b_o"]))
           and not np.any(np.asarray(inputs["b_vup"])))
    key = f"nc{int(nob)}"
    if key not in _CACHE:
        _CACHE[key] = _build(nob)
    _CACHE["nc"] = _CACHE[key]
    nc = _CACHE[key]
    in_maps = _host_prep({k: np.asarray(v) for k, v in inputs.items()})
    res = run_bass_kernel_spmd(nc, in_maps, list(range(NC))).results
    out = np.zeros((B, S, MODEL), np.float32)
    for c in range(NC):
        w = c % G
        o = res[c]["out_sh"].astype(np.float32)          # [NS*128, MODEL]
        for s in range(NS):
            out[c // G, TS * s + 128 * w:TS * s + 128 * (w + 1), :] = \
                o[128 * s:128 * (s + 1), :]
    return out
